# revision 35
# baseline (speedup 1.0000x reference)
# Trainium2 Bass kernel for nn_DVQVAE_Encoder: 6-layer transformer encoder +
# information-weighted segment downsampling + VQ codebook lookup.
# Data-parallel over batch: B=8 rows -> 8 NeuronCores, one row per core.
# Self-contained: builds, compiles and runs the Bass kernel via concourse.
import os
import numpy as np

import concourse.bacc as bacc
import concourse.tile as tile
import concourse.mybir as mybir
from concourse import bass_utils

F32 = mybir.dt.float32
F32R = mybir.dt.float32r
I32 = mybir.dt.int32
U32 = mybir.dt.uint32
AT = mybir.ActivationFunctionType
OP = mybir.AluOpType
AX = mybir.AxisListType

D = 512
NH = 8
DH = 64
FF = 2048
NL = 6
K = 1024
B = 8
T = 1024
SLD = 512
NT = T // 128   # 8 token tiles
ND = D // 128   # 4 feature tiles
NF = FF // 128  # 16 ff tiles
EPS = 1e-5

# matmul groups executed in float32r (TF32-like fast mode, ~1.5e-4 input
# rounding). Empty set = full fp32.
R_GROUPS = set(os.environ.get("KERNEL_R", "").split(",")) - {"", "none"}
DEV_NL = int(os.environ.get("KERNEL_NL", str(NL)))
DEV_NOLN = bool(int(os.environ.get("KERNEL_NOLN", "0")))
DEV_NONORM = bool(int(os.environ.get("KERNEL_NONORM", "0")))

# broadcast-row order in bcrows: embed rows then 7 per layer
_BC_EMB = ["b_emb", "ln_g", "ln_b"]
_BC_LAYER = ["v_bias", "out_b", "ln1_g", "ln1_b", "ff2_b", "ln2_g", "ln2_b"]
N_BCROWS = len(_BC_EMB) + NL * len(_BC_LAYER)


def _pe_table(t, d):
    pos = np.arange(t, dtype=np.float32)[:, None]
    div = np.exp(np.arange(0, d, 2, dtype=np.float32) * (-np.log(10000.0) / d))
    pe = np.zeros((t, d), np.float32)
    pe[:, 0::2] = np.sin(pos * div)
    pe[:, 1::2] = np.cos(pos * div)
    return pe


def build(taps=False, ncores=B, compile=True):
    nc = bacc.Bacc("TRN2", target_bir_lowering=False, debug=False, num_devices=ncores)

    def din(name, shape, dt=F32):
        return nc.dram_tensor(name, shape, dt, kind="ExternalInput").ap()

    def dout(name, shape, dt=F32):
        return nc.dram_tensor(name, shape, dt, kind="ExternalOutput").ap()

    xrow = din("xrow", (T, SLD))
    wembT = din("wembT", (SLD, D))
    petab = din("petab", (T, D))
    wqkT = din("wqkT", (NL, D, 2 * D))
    wvT = din("wvT", (NL, D, D))
    qkvb = din("qkvb", (NL, 128, 12))
    woT = din("woT", (NL, D, D))
    wf1T = din("wf1T", (NL, D, FF))
    f1b = din("f1b", (NL, 128, 16))
    wf2T = din("wf2T", (NL, FF, D))
    bcrows = din("bcrows", (N_BCROWS, D))
    w2T = din("w2T", (D, D))
    w2b = din("w2b", (128, 4))
    w3col = din("w3col", (D, 1))
    w3bn = din("w3bn", (1, 1))  # NEGATED W3_b
    cbT = din("cbT", (D, K))
    cb = din("cb", (K, D))
    cnh = din("cnh", (1, K))    # 0.5*|c_k|^2
    jiota = din("jiota", (1, K))
    cwcols = din("cwcols", (128, 8))
    idn = din("idn", (128, 128))
    ones11 = din("ones11", (1, 1))
    onescol = din("onescol", (128, 1))

    o_zq = dout("o_zq", (T, D))
    o_d = dout("o_d", (T,), I32)
    o_s = dout("o_s", (T,))
    o_z = dout("o_z", (T, D))
    o_i = dout("o_i", (T,))
    o_idx = dout("o_idx", (T,), I32)
    o_h = dout("o_h", (T, D))
    if taps:
        o_x0 = dout("o_x0", (T, D))
        o_x1 = dout("o_x1", (T, D))
        o_hm = dout("o_hm", (D, T))

    from contextlib import ExitStack
    with tile.TileContext(nc) as tc, ExitStack() as top:
        cst = top.enter_context(tc.tile_pool(name="cst", bufs=1))
        bcp = top.enter_context(tc.tile_pool(name="bcp", bufs=3))
        xp = top.enter_context(tc.tile_pool(name="xp", bufs=1))
        xtp = top.enter_context(tc.tile_pool(name="xtp", bufs=1))
        fmp = top.enter_context(tc.tile_pool(name="fmp", bufs=1))
        wkp = top.enter_context(tc.tile_pool(name="wkp", bufs=3))
        colp = top.enter_context(tc.tile_pool(name="colp", bufs=8))
        psp = top.enter_context(tc.tile_pool(name="psp", bufs=2, space="PSUM"))
        psa = top.enter_context(tc.tile_pool(name="psa", bufs=4, space="PSUM"))

        def RB(ap, grp):
            return ap.bitcast(F32R) if grp in R_GROUPS else ap

        def mm(out, lhsT, rhs, start, stop, grp=None):
            nc.tensor.matmul(out, RB(lhsT, grp), RB(rhs, grp), start=start, stop=stop)

        # ---------- constants ----------
        idn_sb = cst.tile([128, 128], F32, tag="idn")
        nc.sync.dma_start(idn_sb[:], idn[:])
        ones11_sb = cst.tile([1, 1], F32, tag="ones11")
        nc.sync.dma_start(ones11_sb[:], ones11[:])
        onescol_sb = cst.tile([128, 1], F32, tag="onescol")
        nc.sync.dma_start(RB(onescol_sb[:], "d"), RB(onescol[:], "d"))

        w3bn_sb = cst.tile([1, 1], F32, tag="w3bn")
        nc.sync.dma_start(w3bn_sb[:], w3bn[:])
        cw_sb = cst.tile([128, 8], F32, tag="cw")
        nc.sync.dma_start(cw_sb[:], cwcols[:])

        _bc_idx = {}
        for i, n in enumerate(_BC_EMB):
            _bc_idx[n] = i
        for l in range(NL):
            for j, n in enumerate(_BC_LAYER):
                _bc_idx[(n, l)] = len(_BC_EMB) + l * len(_BC_LAYER) + j

        def bcast(key):
            i = _bc_idx[key]
            stg = bcp.tile([1, D], F32, tag="bcstg", bufs=2, name=f"bcstg_{i}")
            nc.sync.dma_start(stg[:], bcrows[i:i + 1, :])
            t = bcp.tile([128, D], F32, tag="bc")
            nc.gpsimd.partition_broadcast(t[:], stg[:])
            return t


        # token-major layernorm on a [128, D] tile; src may be PSUM or SBUF.
        # resid: optional residual tile added to src first (sum fused with the
        # mean reduction via tensor_tensor_reduce).
        def layernorm(dst, src, g_bc, b_bc, resid=None):
            if DEV_NOLN:
                nc.vector.tensor_copy(dst, src)
                return
            m = colp.tile([128, 1], F32, tag="c0")
            if resid is not None:
                s2 = wkp.tile([128, D], F32, tag="wk512")
                nc.vector.tensor_tensor(s2[:], src, resid, OP.add)
                src = s2[:]
            nc.vector.tensor_reduce(m[:], src, AX.X, OP.add)
            nc.vector.tensor_scalar_mul(m[:], m[:], 1.0 / D)
            xc = wkp.tile([128, D], F32, tag="wk512")
            nc.vector.tensor_scalar(xc[:], src, m[:], None, OP.subtract)
            v = colp.tile([128, 1], F32, tag="c1")
            sq = wkp.tile([128, D], F32, tag="wk512")
            nc.vector.tensor_tensor(sq[:], xc[:], xc[:], OP.mult)
            nc.vector.tensor_reduce(v[:], sq[:], AX.X, OP.add)
            nc.vector.tensor_scalar(v[:], v[:], 1.0 / D, EPS, OP.mult, OP.add)
            nc.vector.reciprocal(v[:], v[:])
            nc.scalar.activation(v[:], v[:], AT.Sqrt)
            nc.vector.scalar_tensor_tensor(xc[:], xc[:], v[:], g_bc[:],
                                           OP.mult, OP.mult)
            nc.vector.tensor_tensor(dst, xc[:], b_bc[:], OP.add)

        # transpose 8 token-major [128, D] tiles -> 4 feature-major [128, T] tiles
        def transpose_tm_to_fm(xs, grp=None):
            outs = []
            for k in range(ND):
                xt = xtp.tile([128, T], F32, tag=f"xt{k}")
                outs.append(xt)
            for g in range(2):
                for k in range(ND):
                    pt = psa.tile([128, 512], F32, tag="acc")
                    for j in range(4):
                        i = g * 4 + j
                        nc.tensor.transpose(pt[:, j * 128:(j + 1) * 128],
                                            xs[i][:, k * 128:(k + 1) * 128], idn_sb[:])
                    nc.vector.tensor_copy(
                        RB(outs[k][:, g * 512:(g + 1) * 512], grp), pt[:])
            return outs

        # ---------- stage 0: embedding ----------
        xin = []
        for i in range(NT):
            t = wkp.tile([128, SLD], F32, tag="wk512x", bufs=5)
            nc.sync.dma_start(t[:], xrow[i * 128:(i + 1) * 128, :])
            xin.append(t)
        xti = transpose_tm_to_fm(xin, "emb")
        wemb = []
        for k in range(ND):
            t = wkp.tile([128, D], F32, tag="wemb", bufs=4)
            nc.sync.dma_start(RB(t[:], "emb"), RB(wembT[k * 128:(k + 1) * 128, :], "emb"))
            wemb.append(t)
        bemb_bc = bcast("b_emb")
        lng_bc = bcast("ln_g")
        lnb_bc = bcast("ln_b")
        x = []
        for i in range(NT):
            p = psa.tile([128, D], F32, tag="acc")
            for k in range(ND):
                mm(p[:], xti[k][:, i * 128:(i + 1) * 128], wemb[k][:], k == 0, k == ND - 1, "emb")
            s = wkp.tile([128, D], F32, tag="wk512")
            nc.vector.tensor_tensor(s[:], p[:], bemb_bc[:], OP.add)
            xt_ = xp.tile([128, D], F32, tag=f"x{i}")
            layernorm(s[:], s[:], lng_bc, lnb_bc)
            pet = wkp.tile([128, D], F32, tag="wk512x", bufs=5)
            nc.sync.dma_start(pet[:], petab[i * 128:(i + 1) * 128, :])
            # relu(s) + pe
            nc.vector.scalar_tensor_tensor(xt_[:], s[:], 0.0, pet[:], OP.max, OP.add)
            x.append(xt_)
        if taps:
            for i in range(NT):
                nc.sync.dma_start(o_x0[i * 128:(i + 1) * 128, :], x[i][:])

        # ---------- transformer layers ----------
        with ExitStack() as wctx:
            wtp = wctx.enter_context(tc.tile_pool(name="wtp", bufs=1))
            wsp = wctx.enter_context(tc.tile_pool(name="wsp", bufs=2))
            srp = wctx.enter_context(tc.tile_pool(name="srp", bufs=1))
            for l in range(DEV_NL):
                # --- load layer weights
                wqk = []
                for k in range(ND):
                    t = wtp.tile([128, 2 * D], F32, tag=f"wqk{k}")
                    nc.sync.dma_start(RB(t[:], "qkv"), RB(wqkT[l % NL, k * 128:(k + 1) * 128, :], "qkv"))
                    wqk.append(t)
                wv = []
                for k in range(ND):
                    t = wtp.tile([128, D], F32, tag=f"wv{k}")
                    nc.sync.dma_start(RB(t[:], "qkv"), RB(wvT[l % NL, k * 128:(k + 1) * 128, :], "qkv"))
                    wv.append(t)
                wo = []
                for k in range(ND):
                    t = wtp.tile([128, D], F32, tag=f"wo{k}")
                    nc.sync.dma_start(RB(t[:], "out"), RB(woT[l % NL, k * 128:(k + 1) * 128, :], "out"))
                    wo.append(t)
                qb = wsp.tile([128, 12], F32, tag="qb")
                nc.sync.dma_start(qb[:], qkvb[l % NL])
                fb = wsp.tile([128, 16], F32, tag="fb")
                nc.sync.dma_start(fb[:], f1b[l % NL])

                vb_bc = bcast(("v_bias", l % NL))
                ob_bc = bcast(("out_b", l % NL))
                l1g_bc = bcast(("ln1_g", l % NL))
                l1b_bc = bcast(("ln1_b", l % NL))

                # --- x transposed (feature-major)
                xT = transpose_tm_to_fm(x, "qkv")

                # --- Q,K feature-major: qkT[m][p, t] for m in 0..7 (Q: 0-3, K: 4-7)
                qkT = []
                for m in range(8):
                    qt = fmp.tile([128, T], F32, tag=f"qk{m}")
                    qkT.append(qt)
                    pp = [psa.tile([128, 512], F32, tag="acc", name=f"pqk_{l}_{m}_{i2}")
                          for i2 in range(2)]
                    for k in range(ND):
                        for tc2 in range(2):
                            mm(pp[tc2][:], wqk[k][:, m * 128:(m + 1) * 128],
                               xT[k][:, tc2 * 512:(tc2 + 1) * 512], k == 0, k == ND - 1, "qkv")
                    for tc2 in range(2):
                        nc.vector.tensor_scalar_add(
                            RB(qt[:, tc2 * 512:(tc2 + 1) * 512], "attn"), pp[tc2][:], qb[:, m:m + 1])

                # --- V token-major with ones column per head: v_sb[i][p, h*65+ (0..63)]=V, col h*65+64 = 1
                v_sb = []
                for i in range(NT):
                    vt = fmp.tile([128, 8 * 65], F32, tag=f"v{i}")
                    v_sb.append(vt)
                    p = psa.tile([128, D], F32, tag="acc")
                    for k in range(ND):
                        mm(p[:], xT[k][:, i * 128:(i + 1) * 128], wv[k][:], k == 0, k == ND - 1, "qkv")
                    for h in range(NH):
                        nc.vector.tensor_tensor(
                            RB(vt[:, h * 65:h * 65 + 64], "attn"), p[:, h * 64:(h + 1) * 64],
                            vb_bc[:, h * 64:(h + 1) * 64], OP.add)
                    nc.vector.tensor_scalar(RB(vt[:, 64::65], "attn"), cw_sb[:], 0.0, 1.0, OP.mult, OP.add)

                # --- attention per head
                atn = []   # attnT pair tiles [128, T], head pair (2j, 2j+1)
                for j in range(ND):
                    at = xtp.tile([128, T], F32, tag=f"xt{j}")
                    atn.append(at)
                for h in range(NH):
                    base = (h % 2) * 64
                    qt = qkT[h // 2]
                    kt = qkT[4 + h // 2]
                    avt = psp.tile([128, T], F32, tag="big")  # rows 0-63 attn, row 64 sumexp
                    for i in range(NT):
                        exs = []
                        for qc in range(2):
                            stp = psa.tile([128, 512], F32, tag="acc", name=f"stp_{h}_{i}_{qc}")
                            mm(stp[:],
                               kt[base:base + 64, i * 128:(i + 1) * 128],
                               qt[base:base + 64, qc * 512:(qc + 1) * 512], True, True, "attn")
                            ex = wkp.tile([128, 512], F32, tag="wk512", name=f"ex_{h}_{i}_{qc}")
                            nc.scalar.activation(RB(ex[:], "attn"), stp[:], AT.Exp, scale=0.125)
                            exs.append(ex)
                        for qc in range(2):
                            mm(avt[0:65, qc * 512:(qc + 1) * 512],
                               v_sb[i][:, h * 65:(h + 1) * 65],
                               exs[qc][:], i == 0, i == NT - 1, "attn")
                    if DEV_NONORM:
                        nc.vector.tensor_copy(RB(atn[h // 2][(h % 2) * 64:(h % 2) * 64 + 64, :], "out"), avt[0:64, :])
                        continue
                    srow = srp.tile([65, T], F32, tag="srow")
                    nc.vector.tensor_copy(srow[64:65, :], avt[64:65, :])
                    sums0 = srp.tile([1, T], F32, tag="sums0")
                    nc.sync.dma_start(sums0[:], srow[64:65, :])
                    nc.vector.reciprocal(sums0[:], sums0[:])
                    rb = srp.tile([128, T], F32, tag="rb")
                    nc.gpsimd.partition_broadcast(rb[:], sums0[:])
                    if h % 2 == 0:
                        nc.vector.tensor_tensor(RB(atn[h // 2][0:64, :], "out"), avt[0:64, :],
                                                rb[0:64, :], OP.mult)
                    else:
                        tmp = wkp.tile([64, T], F32, tag="odda", bufs=2)
                        nc.vector.tensor_tensor(RB(tmp[:], "out"), avt[0:64, :], rb[0:64, :], OP.mult)
                        nc.sync.dma_start(RB(atn[h // 2][64:128, :], "out"), RB(tmp[:], "out"))

                # --- out projection + residual + LN1 (in-place into x tiles)
                for i in range(NT):
                    p = psa.tile([128, D], F32, tag="acc")
                    for k in range(ND):
                        mm(p[:], atn[k][:, i * 128:(i + 1) * 128], wo[k][:], k == 0, k == ND - 1, "out")
                    s = wkp.tile([128, D], F32, tag="wk512")
                    nc.vector.tensor_tensor(s[:], p[:], ob_bc[:], OP.add)
                    layernorm(x[i][:], s[:], l1g_bc, l1b_bc, resid=x[i][:])
                if taps and l == 0 and DEV_NL >= 1:
                    for i in range(NT):
                        nc.sync.dma_start(o_x1[i * 128:(i + 1) * 128, :], x[i][:])

                f2b_bc = bcast(("ff2_b", l % NL))
                l2g_bc = bcast(("ln2_g", l % NL))
                l2b_bc = bcast(("ln2_b", l % NL))

                # --- FF: xT2, then per t-half: FF1 (feature-major chunks) + FF2 accumulation
                xT2 = transpose_tm_to_fm(x, "ff")
                for th in range(2):
                    pf2 = [psa.tile([128, D], F32, tag="acc", name=f"pf2_{th}_{i2}") for i2 in range(4)]
                    for m in range(NF):
                        wc1t = wtp.tile([128, D], F32, tag="wf1c", bufs=3, name=f"wf1c_{l}_{th}_{m}")
                        for k in range(ND):
                            nc.sync.dma_start(
                                RB(wc1t[:, k * 128:(k + 1) * 128], "ff"),
                                RB(wf1T[l % NL, k * 128:(k + 1) * 128,
                                        m * 128:(m + 1) * 128], "ff"))
                        wc1 = [wc1t[:, k * 128:(k + 1) * 128] for k in range(ND)]
                        wc2 = wsp.tile([128, D], F32, tag="wf2c")
                        nc.sync.dma_start(RB(wc2[:], "ff"), RB(wf2T[l % NL, m * 128:(m + 1) * 128, :], "ff"))
                        pf1 = psp.tile([128, 512], F32, tag="big")
                        for k in range(ND):
                            mm(pf1[:], wc1[k], xT2[k][:, th * 512:(th + 1) * 512],
                               k == 0, k == ND - 1, "ff")
                        fa = wkp.tile([128, 512], F32, tag="wk512")
                        nc.vector.tensor_scalar(RB(fa[:], "ff"), pf1[:], fb[:, m:m + 1], 0.0,
                                                OP.add, OP.max)
                        for t4 in range(4):
                            mm(pf2[t4][:], fa[:, t4 * 128:(t4 + 1) * 128], wc2[:],
                               m == 0, m == NF - 1, "ff")
                    for t4 in range(4):
                        i = th * 4 + t4
                        s = wkp.tile([128, D], F32, tag="wk512")
                        nc.vector.tensor_tensor(s[:], pf2[t4][:], f2b_bc[:], OP.add)
                        layernorm(x[i][:], s[:], l2g_bc, l2b_bc, resid=x[i][:])

        # ---------- stage 7: head, segmentation, VQ ----------
        with ExitStack() as sctx:
            s7 = sctx.enter_context(tc.tile_pool(name="s7", bufs=1))
            # H output
            for i in range(NT):
                nc.sync.dma_start(o_h[i * 128:(i + 1) * 128, :], x[i][:])
            # Hm feature-major
            xT3 = transpose_tm_to_fm(x, "hm")
            w2 = []
            for k in range(ND):
                t = s7.tile([128, D], F32, tag=f"w2{k}")
                nc.sync.dma_start(RB(t[:], "hm"), RB(w2T[k * 128:(k + 1) * 128, :], "hm"))
                w2.append(t)
            w2b_sb = colp.tile([128, 4], F32, tag="w2b")
            nc.sync.dma_start(w2b_sb[:], w2b[:])
            hm = []
            for m in range(ND):
                ht = fmp.tile([128, T], F32, tag=f"qk{m}")
                hm.append(ht)
                pp = [psa.tile([128, 512], F32, tag="acc", name=f"phm_{m}_{i2}")
                      for i2 in range(2)]
                for k in range(ND):
                    for tc2 in range(2):
                        mm(pp[tc2][:], w2[k][:, m * 128:(m + 1) * 128],
                           xT3[k][:, tc2 * 512:(tc2 + 1) * 512], k == 0, k == ND - 1, "hm")
                for tc2 in range(2):
                    sl = ht[:, tc2 * 512:(tc2 + 1) * 512]
                    nc.vector.tensor_scalar(sl, pp[tc2][:], w2b_sb[:, m:m + 1], 0.0, OP.add, OP.max)
                    nc.vector.tensor_tensor(sl, sl, xT3[m][:, tc2 * 512:(tc2 + 1) * 512],
                                            OP.add)
            if taps:
                for m in range(ND):
                    nc.sync.dma_start(o_hm[m * 128:(m + 1) * 128, :], hm[m][:])
            # logits row -> sigmoid -> I_T
            w3_sb = colp.tile([128, 4], F32, tag="w3")
            nc.sync.dma_start(w3_sb[:], w3col[:].rearrange("(a b) c -> b (a c)", b=128))
            plog = psp.tile([1, T], F32, tag="big")
            for tc2 in range(2):
                for k in range(ND):
                    mm(plog[:, tc2 * 512:(tc2 + 1) * 512], w3_sb[:, k:k + 1],
                       hm[k][:, tc2 * 512:(tc2 + 1) * 512], k == 0, k == ND - 1)
            irow = s7.tile([1, T], F32, tag="irow")
            # exp(-(logit + b)) = exp(-logit - b);  bias AP = -W3_b
            nc.scalar.activation(irow[:], plog[:], AT.Exp, bias=w3bn_sb[:], scale=-1.0)
            nc.vector.tensor_scalar_add(irow[:], irow[:], 1.0)
            nc.vector.reciprocal(irow[:], irow[:])
            nc.sync.dma_start(o_i[:].rearrange("(a b) -> a b", a=1), irow[:])
            # cumsum -> floor -> S_T
            crow = s7.tile([1, T], F32, tag="rowtmp", bufs=4, name="crow")
            nc.vector.tensor_tensor_scan(crow[:], irow[:], irow[:], 0.0, OP.add, OP.bypass)
            tirow = s7.tile([1, T], I32, tag="rowtmp", bufs=4, name="tirow")
            nc.vector.tensor_copy(tirow[:], crow[:])
            tfrow = s7.tile([1, T], F32, tag="rowtmp", bufs=4, name="tfrow")
            nc.vector.tensor_copy(tfrow[:], tirow[:])
            gtrow = s7.tile([1, T], F32, tag="rowtmp", bufs=4, name="gtrow")
            nc.vector.tensor_tensor(gtrow[:], tfrow[:], crow[:], OP.is_gt)
            srow_ = s7.tile([1, T], F32, tag="srow_")
            nc.vector.tensor_tensor(srow_[:], tfrow[:], gtrow[:], OP.subtract)
            nc.sync.dma_start(o_s[:].rearrange("(a b) -> a b", a=1), srow_[:])
            # S, I columns (token-major [128,1] per tile)
            scols = colp.tile([128, 8], F32, tag="scols")
            icols = colp.tile([128, 8], F32, tag="icols")
            for i in range(NT):
                pt = psa.tile([128, 1], F32, tag="acc")
                nc.tensor.transpose(pt[:], srow_[:, i * 128:(i + 1) * 128], ones11_sb[:])
                nc.vector.tensor_copy(scols[:, i:i + 1], pt[:])
                pt2 = psa.tile([128, 1], F32, tag="acc")
                nc.tensor.transpose(pt2[:], irow[:, i * 128:(i + 1) * 128], ones11_sb[:])
                nc.vector.tensor_copy(icols[:, i:i + 1], pt2[:])
            # jiota broadcast
            jio_sb = s7.tile([1, K], F32, tag="rowtmp", bufs=4, name="jio_sb")
            nc.sync.dma_start(jio_sb[:], jiota[:])
            jio_bc = s7.tile([128, K], F32, tag="bc1024", bufs=2, name="jio_bc")
            nc.gpsimd.partition_broadcast(jio_bc[:], jio_sb[:])
            # Z = M^T w accumulation (two j-groups of 4 banks) + D counts
            zsb = []
            for j in range(NT):
                zt = fmp.tile([128, D], F32, tag=f"v{j}", name=f"zsb{j}")
                zsb.append(zt)
            pd = psp.tile([1, K], F32, tag="big")
            for jg in range(2):
                pz = [psa.tile([128, D], F32, tag="acc", name=f"pz_{jg}_{i2}") for i2 in range(4)]
                for i in range(NT):
                    mt = wkp.tile([128, K], F32, tag="wk1024")
                    nc.vector.tensor_scalar(RB(mt[:], "d"), jio_bc[:], scols[:, i:i + 1], None,
                                            OP.is_equal)
                    wt_ = wkp.tile([128, D], F32, tag="wk512")
                    nc.vector.tensor_scalar_mul(wt_[:], x[i][:], icols[:, i:i + 1])
                    for j4 in range(4):
                        jj = jg * 4 + j4
                        mm(pz[j4][:], mt[:, jj * 128:(jj + 1) * 128], wt_[:],
                           i == 0, i == NT - 1)
                    # D counts: ones^T M for this j-half
                    mm(pd[:, jg * 512:(jg + 1) * 512], onescol_sb[:],
                       mt[:, jg * 512:(jg + 1) * 512], i == 0, i == NT - 1, "d")
                for j4 in range(4):
                    nc.vector.tensor_copy(zsb[jg * 4 + j4][:], pz[j4][:])
                    nc.sync.dma_start(o_z[(jg * 4 + j4) * 128:(jg * 4 + j4 + 1) * 128, :],
                                      zsb[jg * 4 + j4][:])
            drow = s7.tile([1, K], I32, tag="rowtmp", bufs=4, name="drow")
            nc.vector.tensor_copy(drow[:], pd[:])
            nc.sync.dma_start(o_d[:].rearrange("(a b) -> a b", a=1), drow[:])
            # VQ: G = Z C^T - 0.5|c|^2, argmax
            ct_sb = []
            for k in range(ND):
                t = s7.tile([128, K], F32, tag=f"ct{k}")
                nc.sync.dma_start(RB(t[:], "g"), RB(cbT[k * 128:(k + 1) * 128, :], "g"))
                ct_sb.append(t)
            cnh_sb = s7.tile([1, K], F32, tag="rowtmp", bufs=4, name="cnh_sb")
            nc.sync.dma_start(cnh_sb[:], cnh[:])
            cnh_bc = s7.tile([128, K], F32, tag="bc1024", bufs=2, name="cnh_bc")
            nc.gpsimd.partition_broadcast(cnh_bc[:], cnh_sb[:])
            zT = transpose_tm_to_fm(zsb, "g")
            idxf = colp.tile([128, 8], F32, tag="idxf")
            for i in range(NT):
                pg = psp.tile([128, K], F32, tag="big")
                for k in range(ND):
                    for qc in range(2):
                        mm(pg[:, qc * 512:(qc + 1) * 512],
                           zT[k][:, i * 128:(i + 1) * 128],
                           ct_sb[k][:, qc * 512:(qc + 1) * 512], k == 0, k == ND - 1, "g")
                g = wkp.tile([128, K], F32, tag="wk1024")
                nc.vector.tensor_tensor(g[:], pg[:], cnh_bc[:], OP.subtract)
                mx8 = colp.tile([128, 8], F32, tag="mx8")
                mi8 = colp.tile([128, 8], U32, tag="mi8")
                nc.vector.max_with_indices(mx8[:], mi8[:], g[:])
                nc.sync.dma_start(o_idx[i * 128:(i + 1) * 128].rearrange("(a b) -> a b", b=1),
                                  mi8[:, 0:1].bitcast(I32))
                nc.vector.tensor_copy(idxf[:, i:i + 1], mi8[:, 0:1])
            # idx row + broadcast
            idxrow = s7.tile([1, T], F32, tag="idxrow")
            for i in range(NT):
                pr = psa.tile([1, 128], F32, tag="acc")
                nc.tensor.transpose(pr[:], idxf[:, i:i + 1], idn_sb[:])
                nc.vector.tensor_copy(idxrow[:, i * 128:(i + 1) * 128], pr[:])
            idx_bc = s7.tile([128, T], F32, tag="bc1024", bufs=2, name="idx_bc")
            nc.gpsimd.partition_broadcast(idx_bc[:], idxrow[:])
            # z_q = onehot @ codebook (two ct-groups)
            for ng in range(2):
                pq = [psa.tile([128, D], F32, tag="acc", name=f"pq_{ng}_{i2}") for i2 in range(4)]
                for c8 in range(8):
                    cbt = s7.tile([128, D], F32, tag="cbs", bufs=3, name=f"cbt_{ng}_{c8}")
                    nc.sync.dma_start(cbt[:], cb[c8 * 128:(c8 + 1) * 128, :])
                    oh = wkp.tile([128, T], F32, tag="wk1024")
                    nc.vector.tensor_scalar(oh[:], idx_bc[:], cw_sb[:, c8:c8 + 1], None,
                                            OP.is_equal)
                    for n4 in range(4):
                        nn_ = ng * 4 + n4
                        mm(pq[n4][:], oh[:, nn_ * 128:(nn_ + 1) * 128], cbt[:],
                           c8 == 0, c8 == 7)
                for n4 in range(4):
                    zq = wkp.tile([128, D], F32, tag="wk512")
                    nc.vector.tensor_copy(zq[:], pq[n4][:])
                    nc.sync.dma_start(o_zq[(ng * 4 + n4) * 128:(ng * 4 + n4 + 1) * 128, :],
                                      zq[:])

    if compile:
        nc.compile()
    return nc


_NC_CACHE = {}


def _get_nc(taps=False):
    if taps not in _NC_CACHE:
        _NC_CACHE[taps] = build(taps)
    return _NC_CACHE[taps]


def _host_prep(inputs):
    f = np.float32
    qkv_w = np.asarray(inputs["qkv_w"], f)      # [NL, 3D, D]
    qkv_b = np.asarray(inputs["qkv_b"], f)      # [NL, 3D]
    cbk = np.asarray(inputs["codebook"], f)     # [K, D]
    shared = {
        "wembT": np.ascontiguousarray(np.asarray(inputs["W_emb"], f).T),
        "petab": _pe_table(T, D),
        "wqkT": np.ascontiguousarray(qkv_w[:, :2 * D, :].transpose(0, 2, 1)),
        "wvT": np.ascontiguousarray(qkv_w[:, 2 * D:, :].transpose(0, 2, 1)),
        "qkvb": np.ascontiguousarray(qkv_b.reshape(NL, 12, 128).transpose(0, 2, 1)),
        "woT": np.ascontiguousarray(np.asarray(inputs["out_w"], f).transpose(0, 2, 1)),
        "wf1T": np.ascontiguousarray(np.asarray(inputs["ff1_w"], f).transpose(0, 2, 1)),
        "f1b": np.ascontiguousarray(np.asarray(inputs["ff1_b"], f).reshape(NL, 16, 128)
                                    .transpose(0, 2, 1)),
        "wf2T": np.ascontiguousarray(np.asarray(inputs["ff2_w"], f).transpose(0, 2, 1)),
        "w2T": np.ascontiguousarray(np.asarray(inputs["W2_w"], f).T),
        "w2b": np.ascontiguousarray(np.asarray(inputs["W2_b"], f).reshape(4, 128).T),
        "w3col": np.ascontiguousarray(np.asarray(inputs["W3_w"], f).reshape(D, 1)),
        "w3bn": -np.asarray(inputs["W3_b"], f).reshape(1, 1),
        "cbT": np.ascontiguousarray(cbk.T),
        "cb": cbk,
        "cnh": (0.5 * (cbk.astype(np.float64) ** 2).sum(-1)).astype(f).reshape(1, K),
        "jiota": np.arange(K, dtype=f).reshape(1, K),
        "cwcols": (np.arange(128, dtype=f)[:, None] + 128 * np.arange(8, dtype=f)[None, :]),
        "idn": np.eye(128, dtype=f),
        "ones11": np.ones((1, 1), f),
        "onescol": np.ones((128, 1), f),
    }
    rows = []
    ln_g = np.asarray(inputs["ln_g"], f)
    ln_b = np.asarray(inputs["ln_b"], f)
    rows += [np.asarray(inputs["b_emb"], f), ln_g, ln_b]
    for l in range(NL):
        rows += [qkv_b[l, 2 * D:], np.asarray(inputs["out_b"], f)[l],
                 np.asarray(inputs["ln1_g"], f)[l], np.asarray(inputs["ln1_b"], f)[l],
                 np.asarray(inputs["ff2_b"], f)[l], np.asarray(inputs["ln2_g"], f)[l],
                 np.asarray(inputs["ln2_b"], f)[l]]
    shared["bcrows"] = np.stack(rows)
    return shared


LAST_EXEC_NS = None


def bench_floor(reps=8):
    """Measure the axon/PJRT dispatch floor with a near-empty 8-core kernel."""
    import jax
    import time
    import concourse.tile as tile_
    from jax.sharding import Mesh, PartitionSpec, NamedSharding
    from jax.experimental.shard_map import shard_map
    from concourse import bass2jax, mybir as mb

    nc = bacc.Bacc("TRN2", target_bir_lowering=False, debug=False, num_devices=B)
    xi = nc.dram_tensor("xi", (128, 128), F32, kind="ExternalInput").ap()
    yo = nc.dram_tensor("yo", (128, 128), F32, kind="ExternalOutput").ap()
    with tile_.TileContext(nc) as tc:
        with tc.tile_pool(name="p", bufs=1) as p:
            t = p.tile([128, 128], F32, name="t")
            nc.sync.dma_start(t[:], xi[:])
            nc.vector.tensor_scalar_add(t[:], t[:], 1.0)
            nc.sync.dma_start(yo[:], t[:])
    nc.compile()
    bass2jax.install_neuronx_cc_hook()
    part = nc.partition_id_tensor.name if nc.partition_id_tensor else None
    all_in = ["xi", "yo"] + ([part] if part else [])
    out_avals = [jax.core.ShapedArray((128, 128), np.float32)]

    def _body(*args):
        ops = list(args)
        if part:
            ops.append(bass2jax.partition_id_tensor())
        return tuple(bass2jax._bass_exec_p.bind(
            *ops, out_avals=tuple(out_avals), in_names=tuple(all_in),
            out_names=("yo",), lowering_input_output_aliases=(),
            sim_require_finite=True, sim_require_nnan=True, nc=nc))

    mesh = Mesh(np.asarray(jax.devices()[:B]), ("core",))
    spec = PartitionSpec("core")
    sh = NamedSharding(mesh, spec)
    f = jax.jit(shard_map(_body, mesh=mesh, in_specs=(spec,) * 2,
                          out_specs=(spec,) * 1),
                donate_argnums=(1,), keep_unused=True)
    xin = jax.device_put(np.zeros((B * 128, 128), np.float32), sh)
    ts = []
    for _ in range(reps):
        z = jax.device_put(np.zeros((B * 128, 128), np.float32), sh)
        z.block_until_ready()
        t0 = time.perf_counter()
        o = f(xin, z)
        jax.block_until_ready(o)
        ts.append(time.perf_counter() - t0)
    return [int(t * 1e9) for t in ts]


def bench(inputs, reps=10):
    """Time the compiled NEFF on 8 cores with pre-placed device inputs.

    Replicates bass2jax.run_bass_via_pjrt's multi-core path but keeps inputs
    resident on device so per-call wall time ~= kernel execution time.
    """
    import jax
    import jax.numpy as jnp
    import time
    from jax.sharding import Mesh, PartitionSpec, NamedSharding
    from jax.experimental.shard_map import shard_map
    from concourse import bass2jax, mybir as mb

    nc = _get_nc(taps=False)
    bass2jax.install_neuronx_cc_hook()
    shared = _host_prep(inputs)
    X = np.asarray(inputs["X_T"], np.float32)
    in_maps = [dict(shared, xrow=np.ascontiguousarray(X[b])) for b in range(B)]

    part_name = nc.partition_id_tensor.name if nc.partition_id_tensor else None
    in_names, out_names, out_avals, zero_shapes = [], [], [], []
    for alloc in nc.m.functions[0].allocations:
        if not isinstance(alloc, mb.MemoryLocationSet):
            continue
        name = alloc.memorylocations[0].name
        if alloc.kind == "ExternalInput":
            if name != part_name:
                in_names.append(name)
        elif alloc.kind == "ExternalOutput":
            out_names.append(name)
            shape = tuple(alloc.tensor_shape)
            dtype = mb.dt.np(alloc.dtype)
            out_avals.append(jax.core.ShapedArray(shape, dtype))
            zero_shapes.append((shape, dtype))
    n_params = len(in_names)
    n_outs = len(out_names)
    all_in = list(in_names) + list(out_names)
    if part_name is not None:
        all_in.append(part_name)

    def _body(*args):
        operands = list(args)
        if part_name is not None:
            operands.append(bass2jax.partition_id_tensor())
        outs = bass2jax._bass_exec_p.bind(
            *operands, out_avals=tuple(out_avals), in_names=tuple(all_in),
            out_names=tuple(out_names), lowering_input_output_aliases=(),
            sim_require_finite=True, sim_require_nnan=True, nc=nc)
        return tuple(outs)

    devices = jax.devices()[:B]
    mesh = Mesh(np.asarray(devices), ("core",))
    spec = PartitionSpec("core")
    donate = tuple(range(n_params, n_params + n_outs))
    sharded = jax.jit(
        shard_map(_body, mesh=mesh, in_specs=(spec,) * (n_params + n_outs),
                  out_specs=(spec,) * n_outs, check_rep=False),
        donate_argnums=donate, keep_unused=True)
    sh = NamedSharding(mesh, spec)
    dev_in = [jax.device_put(
        np.concatenate([np.asarray(in_maps[c][nm]) for c in range(B)], axis=0), sh)
        for nm in in_names]

    def zeros():
        return [jax.device_put(np.zeros((B * s[0], *s[1:]), d), sh)
                for (s, d) in zero_shapes]

    times = []
    for rep in range(reps):
        z = zeros()
        for a in z:
            a.block_until_ready()
        t0 = time.perf_counter()
        outs = sharded(*dev_in, *z)
        jax.block_until_ready(outs)
        times.append(time.perf_counter() - t0)
    times_ns = [int(t * 1e9) for t in times]
    return min(times_ns), times_ns


def kernel(**inputs):
    global LAST_EXEC_NS
    nc = _get_nc(taps=False)
    shared = _host_prep(inputs)
    X = np.asarray(inputs["X_T"], np.float32)
    in_maps = []
    for b in range(B):
        m = dict(shared)
        m["xrow"] = np.ascontiguousarray(X[b])
        in_maps.append(m)
    trace = bool(int(os.environ.get("KERNEL_TRACE", "0")))
    res = bass_utils.run_bass_kernel_spmd(nc, in_maps, list(range(B)), trace=trace)
    if res.exec_time_ns is not None:
        LAST_EXEC_NS = res.exec_time_ns
    r = res.results
    z_q = np.stack([r[b]["o_zq"] for b in range(B)])
    d_t = np.stack([r[b]["o_d"] for b in range(B)])
    s_t = np.stack([r[b]["o_s"] for b in range(B)])
    z_t = np.stack([r[b]["o_z"] for b in range(B)])
    i_t = np.stack([r[b]["o_i"] for b in range(B)])
    idx = np.stack([r[b]["o_idx"] for b in range(B)])
    h_t = np.stack([r[b]["o_h"] for b in range(B)])
    return (z_q, d_t, s_t, z_t, i_t, idx, h_t)


# revision 36
# speedup vs baseline: 7.7824x; 7.7824x over previous
# Trainium2 Bass kernel for nn_DVQVAE_Encoder: 6-layer transformer encoder +
# information-weighted segment downsampling + VQ codebook lookup.
# Data-parallel over batch: B=8 rows -> 8 NeuronCores, one row per core.
# Self-contained: builds, compiles and runs the Bass kernel via concourse.
import os
import numpy as np

import concourse.bacc as bacc
import concourse.tile as tile
import concourse.mybir as mybir
from concourse import bass_utils

F32 = mybir.dt.float32
F32R = mybir.dt.float32r
I32 = mybir.dt.int32
U32 = mybir.dt.uint32
AT = mybir.ActivationFunctionType
OP = mybir.AluOpType
AX = mybir.AxisListType

D = 512
NH = 8
DH = 64
FF = 2048
NL = 6
K = 1024
B = 8
T = 1024
SLD = 512
NT = T // 128   # 8 token tiles
ND = D // 128   # 4 feature tiles
NF = FF // 128  # 16 ff tiles
EPS = 1e-5

# matmul groups executed in float32r (TF32-like fast mode, ~1.5e-4 input
# rounding). Empty set = full fp32.
R_GROUPS = set(os.environ.get("KERNEL_R", "").split(",")) - {"", "none"}
DEV_NL = int(os.environ.get("KERNEL_NL", str(NL)))
DEV_NOLN = bool(int(os.environ.get("KERNEL_NOLN", "0")))
DEV_NONORM = bool(int(os.environ.get("KERNEL_NONORM", "0")))

# broadcast-row order in bcrows: embed rows then 7 per layer
_BC_EMB = ["b_emb", "ln_g", "ln_b"]
_BC_LAYER = ["v_bias", "out_b", "ln1_g", "ln1_b", "ff2_b", "ln2_g", "ln2_b"]
N_BCROWS = len(_BC_EMB) + NL * len(_BC_LAYER)


def _pe_table(t, d):
    pos = np.arange(t, dtype=np.float32)[:, None]
    div = np.exp(np.arange(0, d, 2, dtype=np.float32) * (-np.log(10000.0) / d))
    pe = np.zeros((t, d), np.float32)
    pe[:, 0::2] = np.sin(pos * div)
    pe[:, 1::2] = np.cos(pos * div)
    return pe


def build(taps=False, ncores=B, compile=True):
    nc = bacc.Bacc("TRN2", target_bir_lowering=False, debug=False, num_devices=ncores)

    def din(name, shape, dt=F32):
        return nc.dram_tensor(name, shape, dt, kind="ExternalInput").ap()

    def dout(name, shape, dt=F32):
        return nc.dram_tensor(name, shape, dt, kind="ExternalOutput").ap()

    xrow = din("xrow", (T, SLD))
    wembT = din("wembT", (SLD, D))
    petab = din("petab", (T, D))
    wqkT = din("wqkT", (NL, D, 2 * D))
    wvT = din("wvT", (NL, D, D))
    qkvb = din("qkvb", (NL, 128, 12))
    woT = din("woT", (NL, D, D))
    wf1T = din("wf1T", (NL, D, FF))
    f1b = din("f1b", (NL, 128, 16))
    wf2T = din("wf2T", (NL, FF, D))
    bcrows = din("bcrows", (N_BCROWS, D))
    w2T = din("w2T", (D, D))
    w2b = din("w2b", (128, 4))
    w3col = din("w3col", (D, 1))
    w3bn = din("w3bn", (1, 1))  # NEGATED W3_b
    cbT = din("cbT", (D, K))
    cb = din("cb", (K, D))
    cnh = din("cnh", (1, K))    # 0.5*|c_k|^2
    jiota = din("jiota", (1, K))
    cwcols = din("cwcols", (128, 8))
    idn = din("idn", (128, 128))
    ones11 = din("ones11", (1, 1))
    onescol = din("onescol", (128, 1))

    o_zq = dout("o_zq", (T, D))
    o_d = dout("o_d", (T,), I32)
    o_s = dout("o_s", (T,))
    o_z = dout("o_z", (T, D))
    o_i = dout("o_i", (T,))
    o_idx = dout("o_idx", (T,), I32)
    o_h = dout("o_h", (T, D))
    if taps:
        o_x0 = dout("o_x0", (T, D))
        o_x1 = dout("o_x1", (T, D))
        o_hm = dout("o_hm", (D, T))

    from contextlib import ExitStack
    with tile.TileContext(nc) as tc, ExitStack() as top:
        cst = top.enter_context(tc.tile_pool(name="cst", bufs=1))
        bcp = top.enter_context(tc.tile_pool(name="bcp", bufs=3))
        xp = top.enter_context(tc.tile_pool(name="xp", bufs=1))
        xtp = top.enter_context(tc.tile_pool(name="xtp", bufs=1))
        fmp = top.enter_context(tc.tile_pool(name="fmp", bufs=1))
        wkp = top.enter_context(tc.tile_pool(name="wkp", bufs=3))
        colp = top.enter_context(tc.tile_pool(name="colp", bufs=8))
        psp = top.enter_context(tc.tile_pool(name="psp", bufs=2, space="PSUM"))
        psa = top.enter_context(tc.tile_pool(name="psa", bufs=4, space="PSUM"))

        def RB(ap, grp):
            return ap.bitcast(F32R) if grp in R_GROUPS else ap

        def mm(out, lhsT, rhs, start, stop, grp=None):
            nc.tensor.matmul(out, RB(lhsT, grp), RB(rhs, grp), start=start, stop=stop)

        # ---------- constants ----------
        idn_sb = cst.tile([128, 128], F32, tag="idn")
        nc.sync.dma_start(idn_sb[:], idn[:])
        ones11_sb = cst.tile([1, 1], F32, tag="ones11")
        nc.sync.dma_start(ones11_sb[:], ones11[:])
        onescol_sb = cst.tile([128, 1], F32, tag="onescol")
        nc.sync.dma_start(RB(onescol_sb[:], "d"), RB(onescol[:], "d"))

        w3bn_sb = cst.tile([1, 1], F32, tag="w3bn")
        nc.sync.dma_start(w3bn_sb[:], w3bn[:])
        cw_sb = cst.tile([128, 8], F32, tag="cw")
        nc.sync.dma_start(cw_sb[:], cwcols[:])

        _bc_idx = {}
        for i, n in enumerate(_BC_EMB):
            _bc_idx[n] = i
        for l in range(NL):
            for j, n in enumerate(_BC_LAYER):
                _bc_idx[(n, l)] = len(_BC_EMB) + l * len(_BC_LAYER) + j

        def bcast(key):
            i = _bc_idx[key]
            stg = bcp.tile([1, D], F32, tag="bcstg", bufs=2, name=f"bcstg_{i}")
            nc.sync.dma_start(stg[:], bcrows[i:i + 1, :])
            t = bcp.tile([128, D], F32, tag="bc")
            nc.gpsimd.partition_broadcast(t[:], stg[:])
            return t


        # token-major layernorm on a [128, D] tile; src may be PSUM or SBUF.
        # resid: optional residual tile added to src first (sum fused with the
        # mean reduction via tensor_tensor_reduce).
        def layernorm(dst, src, g_bc, b_bc, resid=None):
            if DEV_NOLN:
                nc.vector.tensor_copy(dst, src)
                return
            m = colp.tile([128, 1], F32, tag="c0")
            if resid is not None:
                s2 = wkp.tile([128, D], F32, tag="wk512")
                nc.vector.tensor_tensor(s2[:], src, resid, OP.add)
                src = s2[:]
            nc.vector.tensor_reduce(m[:], src, AX.X, OP.add)
            nc.vector.tensor_scalar_mul(m[:], m[:], 1.0 / D)
            xc = wkp.tile([128, D], F32, tag="wk512")
            nc.vector.tensor_scalar(xc[:], src, m[:], None, OP.subtract)
            v = colp.tile([128, 1], F32, tag="c1")
            sq = wkp.tile([128, D], F32, tag="wk512")
            nc.vector.tensor_tensor(sq[:], xc[:], xc[:], OP.mult)
            nc.vector.tensor_reduce(v[:], sq[:], AX.X, OP.add)
            nc.vector.tensor_scalar(v[:], v[:], 1.0 / D, EPS, OP.mult, OP.add)
            nc.vector.reciprocal(v[:], v[:])
            nc.scalar.activation(v[:], v[:], AT.Sqrt)
            nc.vector.scalar_tensor_tensor(xc[:], xc[:], v[:], g_bc[:],
                                           OP.mult, OP.mult)
            nc.vector.tensor_tensor(dst, xc[:], b_bc[:], OP.add)

        # transpose 8 token-major [128, D] tiles -> 4 feature-major [128, T] tiles
        def transpose_tm_to_fm(xs, grp=None):
            outs = []
            for k in range(ND):
                xt = xtp.tile([128, T], F32, tag=f"xt{k}")
                outs.append(xt)
            for g in range(2):
                for k in range(ND):
                    pt = psa.tile([128, 512], F32, tag="acc")
                    for j in range(4):
                        i = g * 4 + j
                        nc.tensor.transpose(pt[:, j * 128:(j + 1) * 128],
                                            xs[i][:, k * 128:(k + 1) * 128], idn_sb[:])
                    nc.vector.tensor_copy(
                        RB(outs[k][:, g * 512:(g + 1) * 512], grp), pt[:])
            return outs

        # ---------- stage 0: embedding ----------
        xin = []
        for i in range(NT):
            t = wkp.tile([128, SLD], F32, tag="wk512x", bufs=5)
            nc.sync.dma_start(t[:], xrow[i * 128:(i + 1) * 128, :])
            xin.append(t)
        xti = transpose_tm_to_fm(xin, "emb")
        wemb = []
        for k in range(ND):
            t = wkp.tile([128, D], F32, tag="wemb", bufs=4)
            nc.sync.dma_start(RB(t[:], "emb"), RB(wembT[k * 128:(k + 1) * 128, :], "emb"))
            wemb.append(t)
        bemb_bc = bcast("b_emb")
        lng_bc = bcast("ln_g")
        lnb_bc = bcast("ln_b")
        x = []
        for i in range(NT):
            p = psa.tile([128, D], F32, tag="acc")
            for k in range(ND):
                mm(p[:], xti[k][:, i * 128:(i + 1) * 128], wemb[k][:], k == 0, k == ND - 1, "emb")
            s = wkp.tile([128, D], F32, tag="wk512")
            nc.vector.tensor_tensor(s[:], p[:], bemb_bc[:], OP.add)
            xt_ = xp.tile([128, D], F32, tag=f"x{i}")
            layernorm(s[:], s[:], lng_bc, lnb_bc)
            pet = wkp.tile([128, D], F32, tag="wk512x", bufs=5)
            nc.sync.dma_start(pet[:], petab[i * 128:(i + 1) * 128, :])
            # relu(s) + pe
            nc.vector.scalar_tensor_tensor(xt_[:], s[:], 0.0, pet[:], OP.max, OP.add)
            x.append(xt_)
        if taps:
            for i in range(NT):
                nc.sync.dma_start(o_x0[i * 128:(i + 1) * 128, :], x[i][:])

        # ---------- transformer layers ----------
        with ExitStack() as wctx:
            wtp = wctx.enter_context(tc.tile_pool(name="wtp", bufs=1))
            wsp = wctx.enter_context(tc.tile_pool(name="wsp", bufs=2))
            srp = wctx.enter_context(tc.tile_pool(name="srp", bufs=1))
            for l in range(DEV_NL):
                # --- load layer weights
                wqk = []
                for k in range(ND):
                    t = wtp.tile([128, 2 * D], F32, tag=f"wqk{k}")
                    nc.sync.dma_start(RB(t[:], "qkv"), RB(wqkT[l % NL, k * 128:(k + 1) * 128, :], "qkv"))
                    wqk.append(t)
                wv = []
                for k in range(ND):
                    t = wtp.tile([128, D], F32, tag=f"wv{k}")
                    nc.sync.dma_start(RB(t[:], "qkv"), RB(wvT[l % NL, k * 128:(k + 1) * 128, :], "qkv"))
                    wv.append(t)
                wo = []
                for k in range(ND):
                    t = wtp.tile([128, D], F32, tag=f"wo{k}")
                    nc.sync.dma_start(RB(t[:], "out"), RB(woT[l % NL, k * 128:(k + 1) * 128, :], "out"))
                    wo.append(t)
                qb = wsp.tile([128, 12], F32, tag="qb")
                nc.sync.dma_start(qb[:], qkvb[l % NL])
                fb = wsp.tile([128, 16], F32, tag="fb")
                nc.sync.dma_start(fb[:], f1b[l % NL])

                vb_bc = bcast(("v_bias", l % NL))
                ob_bc = bcast(("out_b", l % NL))
                l1g_bc = bcast(("ln1_g", l % NL))
                l1b_bc = bcast(("ln1_b", l % NL))

                # --- x transposed (feature-major)
                xT = transpose_tm_to_fm(x, "qkv")

                # --- Q,K feature-major: qkT[m][p, t] for m in 0..7 (Q: 0-3, K: 4-7)
                qkT = []
                for m in range(8):
                    qt = fmp.tile([128, T], F32, tag=f"qk{m}")
                    qkT.append(qt)
                    pp = [psa.tile([128, 512], F32, tag="acc", name=f"pqk_{l}_{m}_{i2}")
                          for i2 in range(2)]
                    for k in range(ND):
                        for tc2 in range(2):
                            mm(pp[tc2][:], wqk[k][:, m * 128:(m + 1) * 128],
                               xT[k][:, tc2 * 512:(tc2 + 1) * 512], k == 0, k == ND - 1, "qkv")
                    for tc2 in range(2):
                        nc.vector.tensor_scalar_add(
                            RB(qt[:, tc2 * 512:(tc2 + 1) * 512], "attn"), pp[tc2][:], qb[:, m:m + 1])

                # --- V token-major with ones column per head: v_sb[i][p, h*65+ (0..63)]=V, col h*65+64 = 1
                v_sb = []
                for i in range(NT):
                    vt = fmp.tile([128, 8 * 65], F32, tag=f"v{i}")
                    v_sb.append(vt)
                    p = psa.tile([128, D], F32, tag="acc")
                    for k in range(ND):
                        mm(p[:], xT[k][:, i * 128:(i + 1) * 128], wv[k][:], k == 0, k == ND - 1, "qkv")
                    nc.vector.tensor_tensor(
                        RB(vt[:, 0:520].rearrange("p (h c) -> p h c", c=65)[:, :, 0:64], "attn"),
                        p[:].rearrange("p (h c) -> p h c", c=64),
                        vb_bc[:].rearrange("p (h c) -> p h c", c=64), OP.add)
                    nc.vector.tensor_scalar(RB(vt[:, 64::65], "attn"), cw_sb[:], 0.0, 1.0, OP.mult, OP.add)

                # --- attention per head
                atn = []   # attnT pair tiles [128, T], head pair (2j, 2j+1)
                for j in range(ND):
                    at = xtp.tile([128, T], F32, tag=f"xt{j}")
                    atn.append(at)
                for h in range(NH):
                    base = (h % 2) * 64
                    qt = qkT[h // 2]
                    kt = qkT[4 + h // 2]
                    avt = psp.tile([128, T], F32, tag="big")  # rows 0-63 attn, row 64 sumexp
                    for i in range(NT):
                        exs = []
                        for qc in range(2):
                            stp = psa.tile([128, 512], F32, tag="acc", name=f"stp_{h}_{i}_{qc}")
                            mm(stp[:],
                               kt[base:base + 64, i * 128:(i + 1) * 128],
                               qt[base:base + 64, qc * 512:(qc + 1) * 512], True, True, "attn")
                            ex = wkp.tile([128, 512], F32, tag="wk512", name=f"ex_{h}_{i}_{qc}")
                            nc.scalar.activation(RB(ex[:], "attn"), stp[:], AT.Exp, scale=0.125)
                            exs.append(ex)
                        for qc in range(2):
                            mm(avt[0:65, qc * 512:(qc + 1) * 512],
                               v_sb[i][:, h * 65:(h + 1) * 65],
                               exs[qc][:], i == 0, i == NT - 1, "attn")
                    if DEV_NONORM:
                        nc.vector.tensor_copy(RB(atn[h // 2][(h % 2) * 64:(h % 2) * 64 + 64, :], "out"), avt[0:64, :])
                        continue
                    srow = srp.tile([65, T], F32, tag="srow")
                    nc.vector.tensor_copy(srow[64:65, :], avt[64:65, :])
                    sums0 = srp.tile([1, T], F32, tag="sums0")
                    nc.sync.dma_start(sums0[:], srow[64:65, :])
                    nc.vector.reciprocal(sums0[:], sums0[:])
                    rb = srp.tile([128, T], F32, tag="rb")
                    nc.gpsimd.partition_broadcast(rb[:], sums0[:])
                    if h % 2 == 0:
                        nc.vector.tensor_tensor(RB(atn[h // 2][0:64, :], "out"), avt[0:64, :],
                                                rb[0:64, :], OP.mult)
                    else:
                        tmp = wkp.tile([64, T], F32, tag="odda", bufs=2)
                        nc.vector.tensor_tensor(RB(tmp[:], "out"), avt[0:64, :], rb[0:64, :], OP.mult)
                        nc.sync.dma_start(RB(atn[h // 2][64:128, :], "out"), RB(tmp[:], "out"))

                # --- out projection + residual + LN1 (in-place into x tiles)
                for i in range(NT):
                    p = psa.tile([128, D], F32, tag="acc")
                    for k in range(ND):
                        mm(p[:], atn[k][:, i * 128:(i + 1) * 128], wo[k][:], k == 0, k == ND - 1, "out")
                    s = wkp.tile([128, D], F32, tag="wk512")
                    nc.vector.tensor_tensor(s[:], p[:], ob_bc[:], OP.add)
                    layernorm(x[i][:], s[:], l1g_bc, l1b_bc, resid=x[i][:])
                if taps and l == 0 and DEV_NL >= 1:
                    for i in range(NT):
                        nc.sync.dma_start(o_x1[i * 128:(i + 1) * 128, :], x[i][:])

                f2b_bc = bcast(("ff2_b", l % NL))
                l2g_bc = bcast(("ln2_g", l % NL))
                l2b_bc = bcast(("ln2_b", l % NL))

                # --- FF: xT2, then per t-half: FF1 (feature-major chunks) + FF2 accumulation
                xT2 = transpose_tm_to_fm(x, "ff")
                for th in range(2):
                    pf2 = [psa.tile([128, D], F32, tag="acc", name=f"pf2_{th}_{i2}") for i2 in range(4)]
                    for m in range(NF):
                        wc1t = wtp.tile([128, D], F32, tag="wf1c", bufs=3, name=f"wf1c_{l}_{th}_{m}")
                        for k in range(ND):
                            nc.sync.dma_start(
                                RB(wc1t[:, k * 128:(k + 1) * 128], "ff"),
                                RB(wf1T[l % NL, k * 128:(k + 1) * 128,
                                        m * 128:(m + 1) * 128], "ff"))
                        wc1 = [wc1t[:, k * 128:(k + 1) * 128] for k in range(ND)]
                        wc2 = wsp.tile([128, D], F32, tag="wf2c")
                        nc.sync.dma_start(RB(wc2[:], "ff"), RB(wf2T[l % NL, m * 128:(m + 1) * 128, :], "ff"))
                        pf1 = psp.tile([128, 512], F32, tag="big")
                        for k in range(ND):
                            mm(pf1[:], wc1[k], xT2[k][:, th * 512:(th + 1) * 512],
                               k == 0, k == ND - 1, "ff")
                        fa = wkp.tile([128, 512], F32, tag="wk512")
                        nc.vector.tensor_scalar(RB(fa[:], "ff"), pf1[:], fb[:, m:m + 1], 0.0,
                                                OP.add, OP.max)
                        for t4 in range(4):
                            mm(pf2[t4][:], fa[:, t4 * 128:(t4 + 1) * 128], wc2[:],
                               m == 0, m == NF - 1, "ff")
                    for t4 in range(4):
                        i = th * 4 + t4
                        s = wkp.tile([128, D], F32, tag="wk512")
                        nc.vector.tensor_tensor(s[:], pf2[t4][:], f2b_bc[:], OP.add)
                        layernorm(x[i][:], s[:], l2g_bc, l2b_bc, resid=x[i][:])

        # ---------- stage 7: head, segmentation, VQ ----------
        with ExitStack() as sctx:
            s7 = sctx.enter_context(tc.tile_pool(name="s7", bufs=1))
            # H output
            for i in range(NT):
                nc.sync.dma_start(o_h[i * 128:(i + 1) * 128, :], x[i][:])
            # Hm feature-major
            xT3 = transpose_tm_to_fm(x, "hm")
            w2 = []
            for k in range(ND):
                t = s7.tile([128, D], F32, tag=f"w2{k}")
                nc.sync.dma_start(RB(t[:], "hm"), RB(w2T[k * 128:(k + 1) * 128, :], "hm"))
                w2.append(t)
            w2b_sb = colp.tile([128, 4], F32, tag="w2b")
            nc.sync.dma_start(w2b_sb[:], w2b[:])
            hm = []
            for m in range(ND):
                ht = fmp.tile([128, T], F32, tag=f"qk{m}")
                hm.append(ht)
                pp = [psa.tile([128, 512], F32, tag="acc", name=f"phm_{m}_{i2}")
                      for i2 in range(2)]
                for k in range(ND):
                    for tc2 in range(2):
                        mm(pp[tc2][:], w2[k][:, m * 128:(m + 1) * 128],
                           xT3[k][:, tc2 * 512:(tc2 + 1) * 512], k == 0, k == ND - 1, "hm")
                for tc2 in range(2):
                    sl = ht[:, tc2 * 512:(tc2 + 1) * 512]
                    nc.vector.tensor_scalar(sl, pp[tc2][:], w2b_sb[:, m:m + 1], 0.0, OP.add, OP.max)
                    nc.vector.tensor_tensor(sl, sl, xT3[m][:, tc2 * 512:(tc2 + 1) * 512],
                                            OP.add)
            if taps:
                for m in range(ND):
                    nc.sync.dma_start(o_hm[m * 128:(m + 1) * 128, :], hm[m][:])
            # logits row -> sigmoid -> I_T
            w3_sb = colp.tile([128, 4], F32, tag="w3")
            nc.sync.dma_start(w3_sb[:], w3col[:].rearrange("(a b) c -> b (a c)", b=128))
            plog = psp.tile([1, T], F32, tag="big")
            for tc2 in range(2):
                for k in range(ND):
                    mm(plog[:, tc2 * 512:(tc2 + 1) * 512], w3_sb[:, k:k + 1],
                       hm[k][:, tc2 * 512:(tc2 + 1) * 512], k == 0, k == ND - 1)
            irow = s7.tile([1, T], F32, tag="irow")
            # exp(-(logit + b)) = exp(-logit - b);  bias AP = -W3_b
            nc.scalar.activation(irow[:], plog[:], AT.Exp, bias=w3bn_sb[:], scale=-1.0)
            nc.vector.tensor_scalar_add(irow[:], irow[:], 1.0)
            nc.vector.reciprocal(irow[:], irow[:])
            nc.sync.dma_start(o_i[:].rearrange("(a b) -> a b", a=1), irow[:])
            # cumsum -> floor -> S_T
            crow = s7.tile([1, T], F32, tag="rowtmp", bufs=4, name="crow")
            nc.vector.tensor_tensor_scan(crow[:], irow[:], irow[:], 0.0, OP.add, OP.bypass)
            tirow = s7.tile([1, T], I32, tag="rowtmp", bufs=4, name="tirow")
            nc.vector.tensor_copy(tirow[:], crow[:])
            tfrow = s7.tile([1, T], F32, tag="rowtmp", bufs=4, name="tfrow")
            nc.vector.tensor_copy(tfrow[:], tirow[:])
            gtrow = s7.tile([1, T], F32, tag="rowtmp", bufs=4, name="gtrow")
            nc.vector.tensor_tensor(gtrow[:], tfrow[:], crow[:], OP.is_gt)
            srow_ = s7.tile([1, T], F32, tag="srow_")
            nc.vector.tensor_tensor(srow_[:], tfrow[:], gtrow[:], OP.subtract)
            nc.sync.dma_start(o_s[:].rearrange("(a b) -> a b", a=1), srow_[:])
            # S, I columns (token-major [128,1] per tile)
            scols = colp.tile([128, 8], F32, tag="scols")
            icols = colp.tile([128, 8], F32, tag="icols")
            for i in range(NT):
                pt = psa.tile([128, 1], F32, tag="acc")
                nc.tensor.transpose(pt[:], srow_[:, i * 128:(i + 1) * 128], ones11_sb[:])
                nc.vector.tensor_copy(scols[:, i:i + 1], pt[:])
                pt2 = psa.tile([128, 1], F32, tag="acc")
                nc.tensor.transpose(pt2[:], irow[:, i * 128:(i + 1) * 128], ones11_sb[:])
                nc.vector.tensor_copy(icols[:, i:i + 1], pt2[:])
            # jiota broadcast
            jio_sb = s7.tile([1, K], F32, tag="rowtmp", bufs=4, name="jio_sb")
            nc.sync.dma_start(jio_sb[:], jiota[:])
            jio_bc = s7.tile([128, K], F32, tag="bc1024", bufs=2, name="jio_bc")
            nc.gpsimd.partition_broadcast(jio_bc[:], jio_sb[:])
            # Z = M^T w accumulation (two j-groups of 4 banks) + D counts
            zsb = []
            for j in range(NT):
                zt = fmp.tile([128, D], F32, tag=f"v{j}", name=f"zsb{j}")
                zsb.append(zt)
            pd = psp.tile([1, K], F32, tag="big")
            for jg in range(2):
                pz = [psa.tile([128, D], F32, tag="acc", name=f"pz_{jg}_{i2}") for i2 in range(4)]
                for i in range(NT):
                    mt = wkp.tile([128, K], F32, tag="wk1024")
                    nc.vector.tensor_scalar(RB(mt[:], "d"), jio_bc[:], scols[:, i:i + 1], None,
                                            OP.is_equal)
                    wt_ = wkp.tile([128, D], F32, tag="wk512")
                    nc.vector.tensor_scalar_mul(wt_[:], x[i][:], icols[:, i:i + 1])
                    for j4 in range(4):
                        jj = jg * 4 + j4
                        mm(pz[j4][:], mt[:, jj * 128:(jj + 1) * 128], wt_[:],
                           i == 0, i == NT - 1)
                    # D counts: ones^T M for this j-half
                    mm(pd[:, jg * 512:(jg + 1) * 512], onescol_sb[:],
                       mt[:, jg * 512:(jg + 1) * 512], i == 0, i == NT - 1, "d")
                for j4 in range(4):
                    nc.vector.tensor_copy(zsb[jg * 4 + j4][:], pz[j4][:])
                    nc.sync.dma_start(o_z[(jg * 4 + j4) * 128:(jg * 4 + j4 + 1) * 128, :],
                                      zsb[jg * 4 + j4][:])
            drow = s7.tile([1, K], I32, tag="rowtmp", bufs=4, name="drow")
            nc.vector.tensor_copy(drow[:], pd[:])
            nc.sync.dma_start(o_d[:].rearrange("(a b) -> a b", a=1), drow[:])
            # VQ: G = Z C^T - 0.5|c|^2, argmax
            ct_sb = []
            for k in range(ND):
                t = s7.tile([128, K], F32, tag=f"ct{k}")
                nc.sync.dma_start(RB(t[:], "g"), RB(cbT[k * 128:(k + 1) * 128, :], "g"))
                ct_sb.append(t)
            cnh_sb = s7.tile([1, K], F32, tag="rowtmp", bufs=4, name="cnh_sb")
            nc.sync.dma_start(cnh_sb[:], cnh[:])
            cnh_bc = s7.tile([128, K], F32, tag="bc1024", bufs=2, name="cnh_bc")
            nc.gpsimd.partition_broadcast(cnh_bc[:], cnh_sb[:])
            zT = transpose_tm_to_fm(zsb, "g")
            idxf = colp.tile([128, 8], F32, tag="idxf")
            for i in range(NT):
                pg = psp.tile([128, K], F32, tag="big")
                for k in range(ND):
                    for qc in range(2):
                        mm(pg[:, qc * 512:(qc + 1) * 512],
                           zT[k][:, i * 128:(i + 1) * 128],
                           ct_sb[k][:, qc * 512:(qc + 1) * 512], k == 0, k == ND - 1, "g")
                g = wkp.tile([128, K], F32, tag="wk1024")
                nc.vector.tensor_tensor(g[:], pg[:], cnh_bc[:], OP.subtract)
                mx8 = colp.tile([128, 8], F32, tag="mx8")
                mi8 = colp.tile([128, 8], U32, tag="mi8")
                nc.vector.max_with_indices(mx8[:], mi8[:], g[:])
                nc.sync.dma_start(o_idx[i * 128:(i + 1) * 128].rearrange("(a b) -> a b", b=1),
                                  mi8[:, 0:1].bitcast(I32))
                nc.vector.tensor_copy(idxf[:, i:i + 1], mi8[:, 0:1])
            # idx row + broadcast
            idxrow = s7.tile([1, T], F32, tag="idxrow")
            for i in range(NT):
                pr = psa.tile([1, 128], F32, tag="acc")
                nc.tensor.transpose(pr[:], idxf[:, i:i + 1], idn_sb[:])
                nc.vector.tensor_copy(idxrow[:, i * 128:(i + 1) * 128], pr[:])
            idx_bc = s7.tile([128, T], F32, tag="bc1024", bufs=2, name="idx_bc")
            nc.gpsimd.partition_broadcast(idx_bc[:], idxrow[:])
            # z_q = onehot @ codebook (two ct-groups)
            for ng in range(2):
                pq = [psa.tile([128, D], F32, tag="acc", name=f"pq_{ng}_{i2}") for i2 in range(4)]
                for c8 in range(8):
                    cbt = s7.tile([128, D], F32, tag="cbs", bufs=3, name=f"cbt_{ng}_{c8}")
                    nc.sync.dma_start(cbt[:], cb[c8 * 128:(c8 + 1) * 128, :])
                    oh = wkp.tile([128, T], F32, tag="wk1024")
                    nc.vector.tensor_scalar(oh[:], idx_bc[:], cw_sb[:, c8:c8 + 1], None,
                                            OP.is_equal)
                    for n4 in range(4):
                        nn_ = ng * 4 + n4
                        mm(pq[n4][:], oh[:, nn_ * 128:(nn_ + 1) * 128], cbt[:],
                           c8 == 0, c8 == 7)
                for n4 in range(4):
                    zq = wkp.tile([128, D], F32, tag="wk512")
                    nc.vector.tensor_copy(zq[:], pq[n4][:])
                    nc.sync.dma_start(o_zq[(ng * 4 + n4) * 128:(ng * 4 + n4 + 1) * 128, :],
                                      zq[:])

    if compile:
        nc.compile()
    return nc


_NC_CACHE = {}


def _get_nc(taps=False):
    if taps not in _NC_CACHE:
        _NC_CACHE[taps] = build(taps)
    return _NC_CACHE[taps]


def _host_prep(inputs):
    f = np.float32
    qkv_w = np.asarray(inputs["qkv_w"], f)      # [NL, 3D, D]
    qkv_b = np.asarray(inputs["qkv_b"], f)      # [NL, 3D]
    cbk = np.asarray(inputs["codebook"], f)     # [K, D]
    shared = {
        "wembT": np.ascontiguousarray(np.asarray(inputs["W_emb"], f).T),
        "petab": _pe_table(T, D),
        "wqkT": np.ascontiguousarray(qkv_w[:, :2 * D, :].transpose(0, 2, 1)),
        "wvT": np.ascontiguousarray(qkv_w[:, 2 * D:, :].transpose(0, 2, 1)),
        "qkvb": np.ascontiguousarray(qkv_b.reshape(NL, 12, 128).transpose(0, 2, 1)),
        "woT": np.ascontiguousarray(np.asarray(inputs["out_w"], f).transpose(0, 2, 1)),
        "wf1T": np.ascontiguousarray(np.asarray(inputs["ff1_w"], f).transpose(0, 2, 1)),
        "f1b": np.ascontiguousarray(np.asarray(inputs["ff1_b"], f).reshape(NL, 16, 128)
                                    .transpose(0, 2, 1)),
        "wf2T": np.ascontiguousarray(np.asarray(inputs["ff2_w"], f).transpose(0, 2, 1)),
        "w2T": np.ascontiguousarray(np.asarray(inputs["W2_w"], f).T),
        "w2b": np.ascontiguousarray(np.asarray(inputs["W2_b"], f).reshape(4, 128).T),
        "w3col": np.ascontiguousarray(np.asarray(inputs["W3_w"], f).reshape(D, 1)),
        "w3bn": -np.asarray(inputs["W3_b"], f).reshape(1, 1),
        "cbT": np.ascontiguousarray(cbk.T),
        "cb": cbk,
        "cnh": (0.5 * (cbk.astype(np.float64) ** 2).sum(-1)).astype(f).reshape(1, K),
        "jiota": np.arange(K, dtype=f).reshape(1, K),
        "cwcols": (np.arange(128, dtype=f)[:, None] + 128 * np.arange(8, dtype=f)[None, :]),
        "idn": np.eye(128, dtype=f),
        "ones11": np.ones((1, 1), f),
        "onescol": np.ones((128, 1), f),
    }
    rows = []
    ln_g = np.asarray(inputs["ln_g"], f)
    ln_b = np.asarray(inputs["ln_b"], f)
    rows += [np.asarray(inputs["b_emb"], f), ln_g, ln_b]
    for l in range(NL):
        rows += [qkv_b[l, 2 * D:], np.asarray(inputs["out_b"], f)[l],
                 np.asarray(inputs["ln1_g"], f)[l], np.asarray(inputs["ln1_b"], f)[l],
                 np.asarray(inputs["ff2_b"], f)[l], np.asarray(inputs["ln2_g"], f)[l],
                 np.asarray(inputs["ln2_b"], f)[l]]
    shared["bcrows"] = np.stack(rows)
    return shared


LAST_EXEC_NS = None


def bench_floor(reps=8):
    """Measure the axon/PJRT dispatch floor with a near-empty 8-core kernel."""
    import jax
    import time
    import concourse.tile as tile_
    from jax.sharding import Mesh, PartitionSpec, NamedSharding
    from jax.experimental.shard_map import shard_map
    from concourse import bass2jax, mybir as mb

    nc = bacc.Bacc("TRN2", target_bir_lowering=False, debug=False, num_devices=B)
    xi = nc.dram_tensor("xi", (128, 128), F32, kind="ExternalInput").ap()
    yo = nc.dram_tensor("yo", (128, 128), F32, kind="ExternalOutput").ap()
    with tile_.TileContext(nc) as tc:
        with tc.tile_pool(name="p", bufs=1) as p:
            t = p.tile([128, 128], F32, name="t")
            nc.sync.dma_start(t[:], xi[:])
            nc.vector.tensor_scalar_add(t[:], t[:], 1.0)
            nc.sync.dma_start(yo[:], t[:])
    nc.compile()
    bass2jax.install_neuronx_cc_hook()
    part = nc.partition_id_tensor.name if nc.partition_id_tensor else None
    all_in = ["xi", "yo"] + ([part] if part else [])
    out_avals = [jax.core.ShapedArray((128, 128), np.float32)]

    def _body(*args):
        ops = list(args)
        if part:
            ops.append(bass2jax.partition_id_tensor())
        return tuple(bass2jax._bass_exec_p.bind(
            *ops, out_avals=tuple(out_avals), in_names=tuple(all_in),
            out_names=("yo",), lowering_input_output_aliases=(),
            sim_require_finite=True, sim_require_nnan=True, nc=nc))

    mesh = Mesh(np.asarray(jax.devices()[:B]), ("core",))
    spec = PartitionSpec("core")
    sh = NamedSharding(mesh, spec)
    f = jax.jit(shard_map(_body, mesh=mesh, in_specs=(spec,) * 2,
                          out_specs=(spec,) * 1),
                donate_argnums=(1,), keep_unused=True)
    xin = jax.device_put(np.zeros((B * 128, 128), np.float32), sh)
    ts = []
    for _ in range(reps):
        z = jax.device_put(np.zeros((B * 128, 128), np.float32), sh)
        z.block_until_ready()
        t0 = time.perf_counter()
        o = f(xin, z)
        jax.block_until_ready(o)
        ts.append(time.perf_counter() - t0)
    return [int(t * 1e9) for t in ts]


def bench(inputs, reps=10):
    """Time the compiled NEFF on 8 cores with pre-placed device inputs.

    Replicates bass2jax.run_bass_via_pjrt's multi-core path but keeps inputs
    resident on device so per-call wall time ~= kernel execution time.
    """
    import jax
    import jax.numpy as jnp
    import time
    from jax.sharding import Mesh, PartitionSpec, NamedSharding
    from jax.experimental.shard_map import shard_map
    from concourse import bass2jax, mybir as mb

    nc = _get_nc(taps=False)
    bass2jax.install_neuronx_cc_hook()
    shared = _host_prep(inputs)
    X = np.asarray(inputs["X_T"], np.float32)
    in_maps = [dict(shared, xrow=np.ascontiguousarray(X[b])) for b in range(B)]

    part_name = nc.partition_id_tensor.name if nc.partition_id_tensor else None
    in_names, out_names, out_avals, zero_shapes = [], [], [], []
    for alloc in nc.m.functions[0].allocations:
        if not isinstance(alloc, mb.MemoryLocationSet):
            continue
        name = alloc.memorylocations[0].name
        if alloc.kind == "ExternalInput":
            if name != part_name:
                in_names.append(name)
        elif alloc.kind == "ExternalOutput":
            out_names.append(name)
            shape = tuple(alloc.tensor_shape)
            dtype = mb.dt.np(alloc.dtype)
            out_avals.append(jax.core.ShapedArray(shape, dtype))
            zero_shapes.append((shape, dtype))
    n_params = len(in_names)
    n_outs = len(out_names)
    all_in = list(in_names) + list(out_names)
    if part_name is not None:
        all_in.append(part_name)

    def _body(*args):
        operands = list(args)
        if part_name is not None:
            operands.append(bass2jax.partition_id_tensor())
        outs = bass2jax._bass_exec_p.bind(
            *operands, out_avals=tuple(out_avals), in_names=tuple(all_in),
            out_names=tuple(out_names), lowering_input_output_aliases=(),
            sim_require_finite=True, sim_require_nnan=True, nc=nc)
        return tuple(outs)

    devices = jax.devices()[:B]
    mesh = Mesh(np.asarray(devices), ("core",))
    spec = PartitionSpec("core")
    donate = tuple(range(n_params, n_params + n_outs))
    sharded = jax.jit(
        shard_map(_body, mesh=mesh, in_specs=(spec,) * (n_params + n_outs),
                  out_specs=(spec,) * n_outs, check_rep=False),
        donate_argnums=donate, keep_unused=True)
    sh = NamedSharding(mesh, spec)
    dev_in = [jax.device_put(
        np.concatenate([np.asarray(in_maps[c][nm]) for c in range(B)], axis=0), sh)
        for nm in in_names]

    def zeros():
        return [jax.device_put(np.zeros((B * s[0], *s[1:]), d), sh)
                for (s, d) in zero_shapes]

    times = []
    for rep in range(reps):
        z = zeros()
        for a in z:
            a.block_until_ready()
        t0 = time.perf_counter()
        outs = sharded(*dev_in, *z)
        jax.block_until_ready(outs)
        times.append(time.perf_counter() - t0)
    times_ns = [int(t * 1e9) for t in times]
    return min(times_ns), times_ns


def kernel(**inputs):
    global LAST_EXEC_NS
    nc = _get_nc(taps=False)
    shared = _host_prep(inputs)
    X = np.asarray(inputs["X_T"], np.float32)
    in_maps = []
    for b in range(B):
        m = dict(shared)
        m["xrow"] = np.ascontiguousarray(X[b])
        in_maps.append(m)
    trace = bool(int(os.environ.get("KERNEL_TRACE", "0")))
    res = bass_utils.run_bass_kernel_spmd(nc, in_maps, list(range(B)), trace=trace)
    if res.exec_time_ns is not None:
        LAST_EXEC_NS = res.exec_time_ns
    r = res.results
    z_q = np.stack([r[b]["o_zq"] for b in range(B)])
    d_t = np.stack([r[b]["o_d"] for b in range(B)])
    s_t = np.stack([r[b]["o_s"] for b in range(B)])
    z_t = np.stack([r[b]["o_z"] for b in range(B)])
    i_t = np.stack([r[b]["o_i"] for b in range(B)])
    idx = np.stack([r[b]["o_idx"] for b in range(B)])
    h_t = np.stack([r[b]["o_h"] for b in range(B)])
    return (z_q, d_t, s_t, z_t, i_t, idx, h_t)
